# revision 8
# baseline (speedup 1.0000x reference)
"""AANet_influent forward on 8 TRN2 NeuronCores (Bass/Tile).

Math: the reference network has sequence length L=1 everywhere, so every
k=3 conv with pad=1 degenerates to a matmul with the kernel's center tap
(the two pad positions contribute 0), and the k=1 convs are matmuls
directly. The Haar level-1 step is folded into the wave conv weight on
the host. Training-mode BatchNorm needs full-batch statistics: each core
computes per-channel (sum, sumsq) over its local batch shard from the
fp32 PSUM (via bn_stats/bn_aggr), the 8 cores AllGather+reduce the tiny
stat vectors, and BN+bias+ReLU is applied as one fused ACT op
``relu(a*x + c)`` with a = g/sqrt(var+eps), c = beta - a*mean (conv
biases cancel inside BN and are dropped).

Sharding: pure data parallel, batch 16384 -> 2048 per core; all params
replicated. Activations/weights are stored fp16 (PSUM accumulation and
BN statistics stay fp32); measured end-to-end error vs the fp32
reference is ~3e-3 max-rel.
"""
import math
import numpy as np
import ml_dtypes  # noqa: F401  (bf16 dtype registry)

import concourse.bass as bass
import concourse.tile as tile
from concourse import bacc, mybir
from concourse.bass_utils import run_bass_kernel_spmd

F32 = mybir.dt.float32
F16 = mybir.dt.float16
AF = mybir.ActivationFunctionType
ALU = mybir.AluOpType

N_CORES = 8
B = 16384
BL = B // N_CORES          # 2048 per core
CH = 512                   # batch chunk (one PSUM bank of fp32)
NCH = BL // CH             # 4
P = 128

# conv(BN) layers: name -> (cin, cout, src, dst_tag)
#   src names refer to logical activation buffers
CONV = {
    'wave': (128, 128),
    'w1': (130, 256), 'w2': (256, 512),
    'g1': (128, 256), 'g2': (256, 512), 'g3': (512, 1024), 'g4': (1024, 512),
    'g5': (512, 512), 'g6': (512, 1024), 'g7': (1024, 512), 'g8': (512, 512),
    't1': (128, 256), 't2': (256, 512),
    'r1': (512, 512), 'r2': (512, 512),
}
# SBUF slot tag per conv output (tags rotate within pool bufs)
DST_TAG = {
    'wave': 'wf', 'w1': 'act2', 'g1': 'act2', 't1': 'act2',
    'w2': 'act4', 'g2': 'act4', 'r2': 'act4', 'g4': 'act4', 'g5': 'act4',
    'g7': 'act4', 'g8': 'act4', 'r1': 'act4',
    'g3': 'act8', 'g6': 'act8',
    't2': 'tf',
}
# stat sync groups (order matters; layers in a group are independent)
AG_GROUPS = [
    ('wave', 'w1'),
    ('g1', 't1', 'w2'),
    ('g2', 't2'),
    ('g3', 'r2'),
    ('g4',), ('g5',), ('g6',), ('g7',), ('g8',), ('r1',),
]

HEADS = {
    'link': ([512, 256, 128, 64, 32, 2], 'softmax'),
    'net': ([512, 256, 128, 64, 5], 'softmax'),
    'sample': ([512, 256, 128, 64, 32, 2], 'softmax'),
    'infl': ([512, 128, 4], 'none'),
}


def _cdiv(a, b):
    return (a + b - 1) // b


class Builder:
    def __init__(self, nc, tc, pools):
        self.nc = nc
        self.tc = tc
        self.p = pools
        self.acts = {}      # name -> (tile_ap, nt)
        self.wconv = {}     # name -> weight tile
        self.gb = {}        # name -> [128, mt, 2] f32 gamma/beta
        self.ac = {}        # name -> [128, mt, 2] f32 a/c
        self.spack_off = {}
        self.dram = {}
        self._copy_flip = 0

    # ---------- input declarations ----------
    def din(self, name, shape, dt):
        t = self.nc.dram_tensor(name, list(shape), dt, kind="ExternalInput")
        self.dram[name] = t
        return t

    def dout(self, name, shape, dt):
        t = self.nc.dram_tensor(name, list(shape), dt, kind="ExternalOutput")
        self.dram[name] = t
        return t

    # ---------- helpers ----------
    def copy_engine(self):
        self._copy_flip ^= 1
        return self.nc.vector if self._copy_flip else self.nc.scalar

    def load_conv_weight(self, name):
        nc = self.nc
        cin, cout = CONV[name][:2]
        if name == 'w1':
            wa = self.p['wp'].tile([P, 1, 256], F16, tag='wconv', name='w_w1a')
            nc.sync.dma_start(wa[:], self.dram['w_w1a'][:].rearrange(
                "(k p) n -> p k n", p=P))
            wb = self.p['wsm_p'].tile([2, 256], F16, tag='w_w1b', name='w_w1b')
            nc.sync.dma_start(wb[:], self.dram['w_w1b'][:])
            self.wconv[name] = (wa, wb)
            return
        kt = cin // P
        w = self.p['wp'].tile([P, kt, cout], F16, tag='wconv', name=f'w_{name}')
        nc.sync.dma_start(w[:], self.dram[f'w_{name}'][:].rearrange(
            "(k p) n -> p k n", p=P))
        self.wconv[name] = w

    def load_gb(self, name):
        nc = self.nc
        cout = CONV[name][1]
        mt = cout // P
        g = self.p['small'].tile([P, mt, 2], F32, tag=f'gb_{name}', name=f'gb_{name}')
        nc.sync.dma_start(g[:], self.dram[f'gb_{name}'][:].rearrange(
            "(t p) s -> p t s", p=P))
        self.gb[name] = g

    # ---------- conv + stats ----------
    def conv_layer(self, name, src, spack, off):
        """Emit matmuls for all (mtile, chunk), bn stats, raw copies (fp16),
        local (sum, sumsq) written into spack[:, off:off+2*mt]."""
        nc = self.nc
        cin, cout = CONV[name][:2]
        mt = cout // P
        tag = DST_TAG[name]
        tag_bufs = {'act2': 3, 'act4': 2}.get(tag, 1)
        dst = self.p['acts'].tile([P, mt, BL], F16, tag=tag, bufs=tag_bufs,
                                  name=f'a_{name}')
        self.acts[name] = dst

        if name == 'w1':
            wa, wb = self.wconv[name]
            src_lo, src_hi = src
            klist = [(lambda m, _wa=wa: _wa[:, 0, m * P:(m + 1) * P],
                      lambda ch, _s=src_lo: _s[:, 0, ch * CH:(ch + 1) * CH]),
                     (lambda m, _wb=wb: _wb[:, m * P:(m + 1) * P],
                      lambda ch, _s=src_hi: _s[:, ch * CH:(ch + 1) * CH])]
        else:
            w = self.wconv[name]
            kt = cin // P
            klist = [(lambda m, _w=w, _k=k: _w[:, _k, m * P:(m + 1) * P],
                      lambda ch, _s=src, _k=k: _s[:, _k, ch * CH:(ch + 1) * CH])
                     for k in range(kt)]

        st6 = self.p['small'].tile([P, mt, NCH, 6], F32, tag='stats6',
                                   bufs=3, name=f'st6_{name}')
        for m in range(mt):
            for ch in range(NCH):
                ps = self.p['psacc'].tile([P, CH], F32, tag='acc', name='ps')
                nk = len(klist)
                for k, (wfn, sfn) in enumerate(klist):
                    nc.tensor.matmul(ps[:], wfn(m), sfn(ch),
                                     start=(k == 0), stop=(k == nk - 1))
                nc.vector.bn_stats(st6[:, m, ch, :], ps[:])
                eng = self.copy_engine()
                d = dst[:, m, ch * CH:(ch + 1) * CH]
                if eng is self.nc.vector:
                    eng.tensor_copy(d, ps[:])
                else:
                    eng.copy(d, ps[:])

        mv = self.p['small'].tile([P, mt, 2], F32, tag='mv', bufs=2,
                                  name=f'mv_{name}')
        for m in range(mt):
            nc.vector.bn_aggr(mv[:, m, :], st6[:, m, :, :])
        # local sums: sum = mean*BL ; sumsq = (var + mean^2)*BL
        sl = spack[:, off:off + 2 * mt].rearrange("p (m s) -> p m s", s=2)
        t1 = self.p['small'].tile([P, mt], F32, tag='acw', bufs=8, name='t1')
        nc.vector.tensor_mul(t1[:], mv[:, :, 0], mv[:, :, 0])        # mean^2
        nc.vector.tensor_add(t1[:], t1[:], mv[:, :, 1])              # +var
        nc.vector.tensor_scalar_mul(sl[:, :, 1], t1[:], float(BL))   # sumsq
        nc.vector.tensor_scalar_mul(sl[:, :, 0], mv[:, :, 0], float(BL))
        self.spack_off[name] = off
        return off + 2 * mt

    # ---------- AllGather sync ----------
    def ag_sync(self, gi, names, spack, X):
        nc = self.nc
        ccin = self.p['dram'].tile([P, X], F32, tag=f'ccin{gi}', name=f'ccin{gi}')
        ccout = self.p['dram'].tile([N_CORES, P, X], F32, tag=f'ccout{gi}',
                                    addr_space="Shared", name=f'ccout{gi}')
        nc.sync.dma_start(ccin[:], spack[:])
        nc.gpsimd.collective_compute(
            "AllGather", ALU.bypass,
            replica_groups=[list(range(N_CORES))],
            ins=[ccin[:].opt()], outs=[ccout[:].opt()])
        gath = self.p['small'].tile([P, N_CORES, X], F32, tag='gath', bufs=2,
                                    name=f'gath{gi}')
        nc.sync.dma_start(gath[:], ccout[:].rearrange("r p x -> p r x"))
        tot = self.p['small'].tile([P, X], F32, tag='tot', bufs=2,
                                   name=f'tot{gi}')
        nc.vector.tensor_reduce(tot[:], gath[:].rearrange("p r x -> p x r"),
                                axis=mybir.AxisListType.X, op=ALU.add)
        # per-layer a, c
        Bg = float(B)
        for name in names:
            mt = CONV[name][1] // P
            off = self.spack_off[name]
            t3 = tot[:, off:off + 2 * mt].rearrange("p (m s) -> p m s", s=2)
            sm = self.p['small'].tile([P, mt], F32, tag='acw', bufs=8, name='sm')
            nc.vector.tensor_scalar_mul(sm[:], t3[:, :, 0], 1.0 / Bg)   # mean
            sq = self.p['small'].tile([P, mt], F32, tag='acw', bufs=8, name='sq')
            nc.vector.tensor_scalar_mul(sq[:], t3[:, :, 1], 1.0 / Bg)   # E[x^2]
            m2 = self.p['small'].tile([P, mt], F32, tag='acw', bufs=8, name='m2')
            nc.vector.tensor_mul(m2[:], sm[:], sm[:])
            nc.vector.tensor_sub(sq[:], sq[:], m2[:])                   # var
            nc.vector.tensor_scalar_add(sq[:], sq[:], 1e-5)
            nc.scalar.sqrt(sq[:], sq[:])                                # std
            rs = self.p['small'].tile([P, mt], F32, tag='acw', bufs=8, name='rs')
            nc.vector.reciprocal(rs[:], sq[:])
            ac = self.p['small'].tile([P, mt, 2], F32, tag=f'ac_{name}',
                                      name=f'ac_{name}')
            gb = self.gb[name]
            nc.vector.tensor_mul(ac[:, :, 0], gb[:, :, 0], rs[:])      # a
            nc.vector.tensor_mul(rs[:], ac[:, :, 0], sm[:])            # a*mean
            nc.vector.tensor_sub(ac[:, :, 1], gb[:, :, 1], rs[:])      # c
            self.ac[name] = ac

    def normalize(self, name):
        """in-place relu(a*x + c) on the raw fp16 activations."""
        nc = self.nc
        dst = self.acts[name]
        mt = CONV[name][1] // P
        ac = self.ac[name]
        for m in range(mt):
            for ch in range(NCH):
                sl = dst[:, m, ch * CH:(ch + 1) * CH]
                nc.scalar.activation(sl, sl, AF.Relu,
                                     bias=ac[:, m, 1:2], scale=ac[:, m, 0:1])

    def residual_into(self, name, res_src, out_tile=None):
        """out (or res_src buffer) += relu(a*raw + c); raw is acts[name]."""
        nc = self.nc
        raw = self.acts[name]
        mt = CONV[name][1] // P
        ac = self.ac[name]
        target = out_tile if out_tile is not None else res_src
        for m in range(mt):
            for ch in range(NCH):
                tmp = self.p['hp'].tile([P, CH], F16, tag='tmp', bufs=2,
                                        name='tmp')
                nc.scalar.activation(tmp[:], raw[:, m, ch * CH:(ch + 1) * CH],
                                     AF.Relu, bias=ac[:, m, 1:2],
                                     scale=ac[:, m, 0:1])
                sl = target[:, m, ch * CH:(ch + 1) * CH]
                if out_tile is not None:
                    nc.vector.tensor_add(sl, res_src[:, m, ch * CH:(ch + 1) * CH],
                                         tmp[:])
                else:
                    nc.vector.tensor_add(sl, sl, tmp[:])


def build(BL_=BL):
    global BL, NCH
    BL, NCH = BL_, BL_ // CH
    nc = bacc.Bacc(None, target_bir_lowering=False, num_devices=N_CORES)

    bld = None
    # ---- DRAM params ----
    tmp_nc = nc
    din = lambda n, s, d=F16: tmp_nc.dram_tensor(n, list(s), d, kind="ExternalInput")
    dout = lambda n, s, d=F32: tmp_nc.dram_tensor(n, list(s), d, kind="ExternalOutput")

    d = {}
    d['e'] = din('e', [P, BL])
    d['wi_lo'] = din('wi_lo', [P, BL])
    d['wi_hi'] = din('wi_hi', [2, BL])
    for name, (cin, cout) in CONV.items():
        if name == 'w1':
            d['w_w1a'] = din('w_w1a', [128, 256])
            d['w_w1b'] = din('w_w1b', [2, 256])
        else:
            d[f'w_{name}'] = din(f'w_{name}', [cin, cout])
        d[f'gb_{name}'] = din(f'gb_{name}', [cout, 2], F32)
    for hname, (dims, _) in HEADS.items():
        for i in range(len(dims) - 1):
            d[f'w_{hname}{i}'] = din(f'w_{hname}{i}', [dims[i], dims[i + 1]])
            d[f'b_{hname}{i}'] = din(f'b_{hname}{i}',
                                     [min(dims[i + 1], P), _cdiv(dims[i + 1], P)], F32)
    d['w_wsm'] = din('w_wsm', [512, 2])
    d['b_wsm'] = din('b_wsm', [2, 1], F32)

    d['o_feat'] = dout('o_feat', [P, 4, BL], F16)
    d['o_link'] = dout('o_link', [2, BL])
    d['o_net'] = dout('o_net', [5, BL])
    d['o_sample'] = dout('o_sample', [2, BL])
    d['o_infl'] = dout('o_infl', [4, BL])

    with tile.TileContext(nc) as tc:
        import contextlib
        with contextlib.ExitStack() as ctx:
            pools = {
                'acts': ctx.enter_context(tc.tile_pool(name='acts', bufs=1)),
                'wp': ctx.enter_context(tc.tile_pool(name='wp', bufs=2)),
                'wh': ctx.enter_context(tc.tile_pool(name='wh', bufs=1)),
                'small': ctx.enter_context(tc.tile_pool(name='small', bufs=1)),
                'hp': ctx.enter_context(tc.tile_pool(name='hp', bufs=2)),
                'psacc': ctx.enter_context(
                    tc.tile_pool(name='psacc', bufs=5, space='PSUM')),
                'psb': ctx.enter_context(
                    tc.tile_pool(name='psb', bufs=2, space='PSUM')),
                'psd': ctx.enter_context(
                    tc.tile_pool(name='psd', bufs=1, space='PSUM')),
                'dram': ctx.enter_context(tc.tile_pool(name='dram', bufs=1,
                                                       space='DRAM')),
                'wsm_p': ctx.enter_context(tc.tile_pool(name='wsm_p', bufs=1)),
            }
            b = Builder(nc, tc, pools)
            b.dram = d
            _build_body(b)
    nc.compile()
    return nc


def _build_body(b):
    nc = b.nc
    pools = b.p
    small, acts, hp = pools['small'], pools['acts'], pools['hp']

    # ---- constants ----
    ones_c = small.tile([8, 1], F32, tag='ones_c', name='ones_c')
    nc.gpsimd.memset(ones_c[:], 1.0)
    ones_r16 = small.tile([1, P], F16, tag='ones_r16', name='ones_r16')
    nc.gpsimd.memset(ones_r16[:], 1.0)
    ones_r32 = small.tile([1, P], F32, tag='ones_r32', name='ones_r32')
    nc.gpsimd.memset(ones_r32[:], 1.0)

    # ---- head weights + biases (resident) ----
    whead = {}
    bhead = {}
    for hname, (dims, _) in HEADS.items():
        for i in range(len(dims) - 1):
            cin, cout = dims[i], dims[i + 1]
            kt = _cdiv(cin, P)
            if cin >= P:
                w = pools['wh'].tile([P, kt, cout], F16, tag=f'wh_{hname}{i}',
                                     name=f'wh_{hname}{i}')
                nc.sync.dma_start(w[:], b.dram[f'w_{hname}{i}'][:].rearrange(
                    "(k p) n -> p k n", p=P))
            else:
                w = pools['wh'].tile([cin, cout], F16, tag=f'wh_{hname}{i}',
                                     name=f'wh_{hname}{i}')
                nc.sync.dma_start(w[:], b.dram[f'w_{hname}{i}'][:])
            whead[(hname, i)] = w
            bb = pools['wh'].tile([min(cout, P), _cdiv(cout, P)], F32,
                                  tag=f'bh_{hname}{i}', name=f'bh_{hname}{i}')
            nc.sync.dma_start(bb[:], b.dram[f'b_{hname}{i}'][:])
            bhead[(hname, i)] = bb
    w_wsm = pools['wh'].tile([P, 4, 2], F16, tag='w_wsm', name='w_wsm')
    nc.sync.dma_start(w_wsm[:], b.dram['w_wsm'][:].rearrange(
        "(k p) n -> p k n", p=P))
    b_wsm = pools['wh'].tile([2, 1], F32, tag='b_wsm', name='b_wsm')
    nc.sync.dma_start(b_wsm[:], b.dram['b_wsm'][:])

    # ---- gamma/beta tables ----
    for name in CONV:
        b.load_gb(name)

    # ---- inputs ----
    e = acts.tile([P, 1, BL], F16, tag='ex', name='e')
    nc.sync.dma_start(e[:, 0, :], b.dram['e'][:])
    wi_lo = acts.tile([P, 1, BL], F16, tag='wi', name='wi_lo')
    nc.sync.dma_start(wi_lo[:, 0, :], b.dram['wi_lo'][:])
    wi_hi = acts.tile([2, BL], F16, tag='wihi', name='wi_hi')
    nc.sync.dma_start(wi_hi[:], b.dram['wi_hi'][:])

    # ================= group 1: wave + w1 =================
    spgrp = {}

    def new_spack(gi, X):
        sp = small.tile([P, X], F32, tag='spack', bufs=2, name=f'spack{gi}')
        spgrp[gi] = sp
        return sp

    def group_X(names):
        return sum(2 * (CONV[n][1] // P) for n in names)

    gi = 0
    names = AG_GROUPS[gi]
    X = group_X(names)
    sp = new_spack(gi, X)
    b.load_conv_weight('wave')
    off = b.conv_layer('wave', e, sp, 0)
    b.load_conv_weight('w1')
    off = b.conv_layer('w1', (wi_lo, wi_hi), sp, off)
    b.ag_sync(gi, names, sp, X)
    b.normalize('wave')
    # x = e + 0.3*wf  (in place into e)
    wf = b.acts['wave']
    nc.vector.scalar_tensor_tensor(e[:, 0, :], wf[:, 0, :], 0.3, e[:, 0, :],
                                   op0=ALU.mult, op1=ALU.add)
    x = e
    b.normalize('w1')

    # ================= group 2: g1 t1 w2 =================
    gi = 1
    names = AG_GROUPS[gi]
    X = group_X(names)
    sp = new_spack(gi, X)
    b.load_conv_weight('g1')
    off = b.conv_layer('g1', x, sp, 0)
    b.load_conv_weight('t1')
    off = b.conv_layer('t1', x, sp, off)
    b.load_conv_weight('w2')
    off = b.conv_layer('w2', b.acts['w1'], sp, off)
    b.ag_sync(gi, names, sp, X)
    b.normalize('g1')
    b.normalize('t1')
    b.normalize('w2')

    # ---- wsm head: store exp(logits) [2, BL] f32; softmax folded into fusion ----
    ez_all = acts.tile([2, BL], F32, tag='wo', name='ez_all')
    w2n = b.acts['w2']
    for ch in range(NCH):
        csl = slice(ch * CH, (ch + 1) * CH)
        ps = pools['psacc'].tile([2, CH], F32, tag='acc', name='ps_wsm')
        for k in range(4):
            nc.tensor.matmul(ps[:], w_wsm[:, k, :], w2n[:, k, csl],
                             start=(k == 0), stop=(k == 3))
        nc.scalar.activation(ez_all[:, csl], ps[:], AF.Exp,
                             bias=b_wsm[:, 0:1], scale=1.0)

    # ================= group 3: g2 t2 =================
    gi = 2
    names = AG_GROUPS[gi]
    X = group_X(names)
    sp = new_spack(gi, X)
    b.load_conv_weight('g2')
    off = b.conv_layer('g2', b.acts['g1'], sp, 0)
    b.load_conv_weight('t2')
    off = b.conv_layer('t2', b.acts['t1'], sp, off)
    b.ag_sync(gi, names, sp, X)
    b.normalize('g2')
    b.normalize('t2')          # -> tf

    # ================= group 4: g3 ; r2 =================
    gi = 3
    names = AG_GROUPS[gi]
    X = group_X(names)
    sp = new_spack(gi, X)
    b.load_conv_weight('g3')
    off = b.conv_layer('g3', b.acts['g2'], sp, 0)
    b.load_conv_weight('r2')
    off = b.conv_layer('r2', b.acts['t2'], sp, off)
    b.ag_sync(gi, names, sp, X)
    b.normalize('g3')
    # t_final = tf + relu(a*r2 + c)   (in place into tf)
    b.residual_into('r2', b.acts['t2'])
    tfin = b.acts['t2']

    # ================= groups 5..9: g4..g8 =================
    prev = 'g3'
    for gi, name in zip(range(4, 9), ['g4', 'g5', 'g6', 'g7', 'g8']):
        X = group_X((name,))
        sp = new_spack(gi, X)
        b.load_conv_weight(name)
        b.conv_layer(name, b.acts[prev], sp, 0)
        b.ag_sync(gi, (name,), sp, X)
        b.normalize(name)
        prev = name
    gf = b.acts['g8']

    # ================= group 10: r1 =================
    gi = 9
    X = group_X(('r1',))
    sp = new_spack(gi, X)
    b.load_conv_weight('r1')
    b.conv_layer('r1', gf, sp, 0)
    b.ag_sync(gi, ('r1',), sp, X)
    # feat = gf + relu(a*r1 + c)  (into fresh feat tile)
    feat = acts.tile([P, 4, BL], F16, tag='feat', name='feat')
    b.residual_into('r1', gf, out_tile=feat)

    # ================= fusion + heads per chunk =================
    ones2 = small.tile([2, P], F32, tag='ones2', name='ones2')
    nc.gpsimd.memset(ones2[:], 1.0)
    for ch in range(NCH):
        csl = slice(ch * CH, (ch + 1) * CH)
        # wo0 broadcast = e0/(e0+e1) per sample, replicated over partitions
        bc0 = pools['psb'].tile([P, CH], F32, tag='bc', name='bc0')
        nc.tensor.matmul(bc0[:], ones_r32[0:1, :], ez_all[0:1, csl],
                         start=True, stop=True)
        bcd = pools['psb'].tile([P, CH], F32, tag='bc', name='bcd')
        nc.tensor.matmul(bcd[:], ones2[:], ez_all[:, csl],
                         start=True, stop=True)
        wrec = hp.tile([P, CH], F32, tag='wrec', bufs=2, name='wrec')
        nc.vector.reciprocal(wrec[:], bcd[:])
        wo0b = hp.tile([P, CH], F32, tag='wo0b', bufs=2, name='wo0b')
        nc.vector.tensor_mul(wo0b[:], wrec[:], bc0[:])
        # feature = tf + wo0*(gfinal - tf)   (wo1 == 1-wo0 for 2-way softmax)
        for m in range(4):
            fsl = feat[:, m, csl]
            tsl = tfin[:, m, csl]
            nc.vector.tensor_sub(fsl, fsl, tsl)
            nc.vector.scalar_tensor_tensor(fsl, fsl, 1.0, wo0b[:],
                                           op0=ALU.mult, op1=ALU.mult)
            nc.vector.tensor_add(fsl, fsl, tsl)

        # ---- heads on this chunk ----
        for hname, (dims, final) in HEADS.items():
            cur = feat
            cur_sl = csl
            nlay = len(dims) - 1
            for i in range(nlay):
                cin, cout = dims[i], dims[i + 1]
                kt = _cdiv(cin, P)
                mt = _cdiv(cout, P)
                w = whead[(hname, i)]
                bb = bhead[(hname, i)]
                last = (i == nlay - 1)
                if not last:
                    h = hp.tile([min(cout, P), mt, CH], F16,
                                tag=f'h{min(cout, P)}_{mt}', bufs=2,
                                name=f'h_{hname}{i}')
                for m in range(mt):
                    pp = min(P, cout - m * P)
                    ps = pools['psacc'].tile([pp, CH], F32, tag='acc',
                                             name=f'ps_{hname}{i}')
                    for k in range(kt):
                        if cin >= P:
                            wsl = w[:, k, m * P:m * P + pp]
                            xsl = cur[:, k, cur_sl]
                        else:
                            wsl = w[:, m * P:m * P + pp]
                            xsl = cur[:, 0, cur_sl] if cur.shape[1] > 1 else \
                                cur[:, cur_sl] if len(cur.shape) == 2 else \
                                cur[:, 0, cur_sl]
                        nc.tensor.matmul(ps[:], wsl, xsl,
                                         start=(k == 0), stop=(k == kt - 1))
                    if not last:
                        nc.scalar.activation(h[:pp, m, :], ps[:], AF.Relu,
                                             bias=bb[:pp, m:m + 1], scale=1.0)
                    else:
                        if final == 'softmax':
                            M = cout
                            ez = hp.tile([M, CH], F32, tag='ez', bufs=1,
                                         name=f'ez_{hname}')
                            nc.scalar.activation(ez[:], ps[:], AF.Exp,
                                                 bias=bb[:M, 0:1], scale=1.0)
                            den = pools['psd'].tile([1, CH], F32, tag='den',
                                                    name='den_h')
                            nc.tensor.matmul(den[:], ones_c[0:M, 0:1], ez[:],
                                             start=True, stop=True)
                            rec = hp.tile([1, CH], F32, tag='rec', bufs=1,
                                          name='rec_h')
                            nc.vector.reciprocal(rec[:], den[:])
                            bcp = pools['psb'].tile([M, CH], F32, tag='bc',
                                                    name='bc_h')
                            nc.tensor.matmul(bcp[:], ones_r32[0:1, 0:M], rec[:],
                                             start=True, stop=True)
                            sm = hp.tile([M, CH], F32, tag='sm', bufs=1,
                                         name=f'sm_{hname}')
                            nc.vector.tensor_mul(sm[:], ez[:], bcp[:])
                            nc.sync.dma_start(b.dram[f'o_{hname}'][:, csl], sm[:])
                        else:
                            oo = hp.tile([cout, CH], F32, tag='io', bufs=1,
                                         name=f'io_{hname}')
                            nc.scalar.activation(oo[:], ps[:], AF.Identity,
                                                 bias=bb[:cout, 0:1], scale=1.0)
                            nc.sync.dma_start(b.dram[f'o_{hname}'][:, csl], oo[:])
                if not last:
                    cur = h
                    cur_sl = slice(0, CH)

    # feature out
    for m in range(4):
        nc.sync.dma_start(b.dram['o_feat'][:, m, :], feat[:, m, :])


# ======================= host side =======================

def _prep_inputs(inputs):
    """Fold/transposes/shard on the host; returns in_maps list."""
    f16 = np.float16

    def npf(x):
        return np.asarray(x, dtype=np.float32)

    e = npf(inputs['node_embedding'])[:, 0, :]      # [B, 128]
    wi = npf(inputs['weight_input'])[:, 0, :]       # [B, 130]
    IN = e.shape[1]

    H = np.zeros((IN // 2, IN), np.float32)
    for j in range(IN // 2):
        H[j, 2 * j] = H[j, 2 * j + 1] = 1.0 / np.sqrt(2.0, dtype=np.float32)

    conv_params = {
        'wave': inputs['wave_params'][0],
        'w1': inputs['weight_params'][0], 'w2': inputs['weight_params'][1],
        'g1': inputs['gen_params'][0], 'g2': inputs['gen_params'][1],
        'g3': inputs['gen_params'][2], 'g4': inputs['gen_params'][3],
        'g5': inputs['gen_params'][4], 'g6': inputs['gen_params'][5],
        'g7': inputs['gen_params'][6], 'g8': inputs['gen_params'][7],
        't1': inputs['tgt_params'][0], 't2': inputs['tgt_params'][1],
        'r1': inputs['res1_params'][0], 'r2': inputs['res2_params'][0],
    }

    shared = {}
    for name, (W, bias, g, beta) in conv_params.items():
        W = npf(W)
        k = W.shape[-1]
        Weff = W[:, :, (k - 1) // 2]                 # [cout, cin]
        if name == 'wave':
            Weff = Weff @ H                          # fold Haar
        lhsT = np.ascontiguousarray(Weff.T).astype(f16)   # [cin, cout]
        if name == 'w1':
            shared['w_w1a'] = lhsT[:128]
            shared['w_w1b'] = lhsT[128:130]
        else:
            shared[f'w_{name}'] = lhsT
        shared[f'gb_{name}'] = np.ascontiguousarray(
            np.stack([npf(g), npf(beta)], axis=1))   # [cout, 2]

    head_params = {
        'link': inputs['link_mlp'], 'net': inputs['net_mlp'],
        'sample': inputs['sample_mlp'], 'infl': inputs['infl_mlp'],
    }
    for hname, params in head_params.items():
        for i, (W, bias) in enumerate(params):
            W = npf(W)
            shared[f'w_{hname}{i}'] = np.ascontiguousarray(W.T).astype(f16)
            cout = W.shape[0]
            pp = min(cout, P)
            nt = _cdiv(cout, P)
            bp = np.zeros((pp, nt), np.float32)
            bias = npf(bias)
            for t in range(nt):
                seg = bias[t * P:(t + 1) * P]
                bp[:len(seg), t] = seg
            shared[f'b_{hname}{i}'] = bp
    Wsm, bsm = inputs['wsm_params'][0]
    shared['w_wsm'] = np.ascontiguousarray(npf(Wsm).T).astype(f16)
    shared['b_wsm'] = npf(bsm).reshape(2, 1).copy()

    in_maps = []
    for c in range(N_CORES):
        sl = slice(c * BL, (c + 1) * BL)
        m = dict(shared)
        m['e'] = np.ascontiguousarray(e[sl].T).astype(f16)
        wi_c = wi[sl]
        m['wi_lo'] = np.ascontiguousarray(wi_c[:, :128].T).astype(f16)
        m['wi_hi'] = np.ascontiguousarray(wi_c[:, 128:130].T).astype(f16)
        in_maps.append(m)
    return in_maps


_CACHED_NC = None
TRACE = False
LAST_RESULTS = None


def kernel(**inputs):
    global _CACHED_NC, LAST_RESULTS
    in_maps = _prep_inputs(inputs)
    if _CACHED_NC is None:
        _CACHED_NC = build()
    nc = _CACHED_NC
    res = run_bass_kernel_spmd(nc, in_maps, core_ids=list(range(N_CORES)),
                               trace=TRACE)
    LAST_RESULTS = res

    link = np.zeros((B, 2), np.float32)
    net = np.zeros((B, 5), np.float32)
    sample = np.zeros((B, 2), np.float32)
    feature = np.zeros((B, 512), np.float32)
    infl = np.zeros((B, 4), np.float32)
    for c, out in enumerate(res.results):
        sl = slice(c * BL, (c + 1) * BL)
        link[sl] = out['o_link'].T
        net[sl] = out['o_net'].T
        sample[sl] = out['o_sample'].T
        infl[sl] = out['o_infl'].T
        f = out['o_feat'].astype(np.float32)          # [128, 4, BL]
        feature[sl] = f.transpose(2, 1, 0).reshape(BL, 512)
    return (link, net, sample, feature, infl)


if __name__ == '__main__':
    nc = build()
    print("built ok")


# revision 11
# speedup vs baseline: 1.2945x; 1.2945x over previous
"""AANet_influent forward on 8 TRN2 NeuronCores (Bass/Tile).

Math: the reference network has sequence length L=1 everywhere, so every
k=3 conv with pad=1 degenerates to a matmul with the kernel's center tap
(the two pad positions contribute 0), and the k=1 convs are matmuls
directly. The Haar level-1 step is folded into the wave conv weight on
the host. Training-mode BatchNorm needs full-batch statistics: each core
computes per-channel (sum, sumsq) over its local batch shard from the
fp32 PSUM (via bn_stats/bn_aggr), the 8 cores AllGather+reduce the tiny
stat vectors, and BN+bias+ReLU is applied as one fused ACT op
``relu(a*x + c)`` with a = g/sqrt(var+eps), c = beta - a*mean (conv
biases cancel inside BN and are dropped).

Sharding: pure data parallel, batch 16384 -> 2048 per core; all params
replicated. Activations/weights are stored fp16 (PSUM accumulation and
BN statistics stay fp32); measured end-to-end error vs the fp32
reference is ~3e-3 max-rel.
"""
import math
import numpy as np
import ml_dtypes  # noqa: F401  (bf16 dtype registry)

import concourse.bass as bass
import concourse.tile as tile
from concourse import bacc, mybir
from concourse.bass_utils import run_bass_kernel_spmd

F32 = mybir.dt.float32
F16 = mybir.dt.float16
AF = mybir.ActivationFunctionType
ALU = mybir.AluOpType

N_CORES = 8
B = 16384
BL = B // N_CORES          # 2048 per core
CH = 512                   # batch chunk (one PSUM bank of fp32)
NCH = BL // CH             # 4
P = 128

# conv(BN) layers: name -> (cin, cout, src, dst_tag)
#   src names refer to logical activation buffers
CONV = {
    'wave': (128, 128),
    'w1': (130, 256), 'w2': (256, 512),
    'g1': (128, 256), 'g2': (256, 512), 'g3': (512, 1024), 'g4': (1024, 512),
    'g5': (512, 512), 'g6': (512, 1024), 'g7': (1024, 512), 'g8': (512, 512),
    't1': (128, 256), 't2': (256, 512),
    'r1': (512, 512), 'r2': (512, 512),
}
# SBUF slot tag per conv output (tags rotate within pool bufs)
DST_TAG = {
    'wave': 'wf', 'w1': 'act2', 'g1': 'act2', 't1': 'act2',
    'w2': 'act4', 'g2': 'act4', 'r2': 'act4', 'g4': 'act4', 'g5': 'act4',
    'g7': 'act4', 'g8': 'act4', 'r1': 'act4',
    'g3': 'act8', 'g6': 'act8',
    't2': 'tf',
}
# stat sync groups (order matters; layers in a group are independent)
AG_GROUPS = [
    ('wave', 'w1'),
    ('g1', 't1', 'w2'),
    ('g2', 't2'),
    ('g3', 'r2'),
    ('g4',), ('g5',), ('g6',), ('g7',), ('g8',), ('r1',),
]

HEADS = {
    'link': ([512, 256, 128, 64, 32, 2], 'sigmoid2'),
    'net': ([512, 256, 128, 64, 5], 'softmax'),
    'sample': ([512, 256, 128, 64, 32, 2], 'sigmoid2'),
    'infl': ([512, 128, 4], 'none'),
}


def _cdiv(a, b):
    return (a + b - 1) // b


class Builder:
    def __init__(self, nc, tc, pools):
        self.nc = nc
        self.tc = tc
        self.p = pools
        self.acts = {}      # name -> (tile_ap, nt)
        self.wconv = {}     # name -> weight tile
        self.gb = {}        # name -> [128, mt, 2] f32 gamma/beta
        self.ac = {}        # name -> [128, mt, 2] f32 a/c
        self.spack_off = {}
        self.dram = {}
        self._copy_flip = 0

    # ---------- input declarations ----------
    def din(self, name, shape, dt):
        t = self.nc.dram_tensor(name, list(shape), dt, kind="ExternalInput")
        self.dram[name] = t
        return t

    def dout(self, name, shape, dt):
        t = self.nc.dram_tensor(name, list(shape), dt, kind="ExternalOutput")
        self.dram[name] = t
        return t

    # ---------- helpers ----------
    def copy_engine(self):
        self._copy_flip ^= 1
        return self.nc.vector if self._copy_flip else self.nc.scalar

    def load_conv_weight(self, name):
        nc = self.nc
        cin, cout = CONV[name][:2]
        if name == 'w1':
            wa = self.p['wp'].tile([P, 1, 256], F16, tag='wconv', name='w_w1a')
            nc.sync.dma_start(wa[:], self.dram['w_w1a'][:].rearrange(
                "(k p) n -> p k n", p=P))
            wb = self.p['wsm_p'].tile([2, 256], F16, tag='w_w1b', name='w_w1b')
            nc.sync.dma_start(wb[:], self.dram['w_w1b'][:])
            self.wconv[name] = (wa, wb)
            return
        kt = cin // P
        w = self.p['wp'].tile([P, kt, cout], F16, tag='wconv', name=f'w_{name}')
        nc.sync.dma_start(w[:], self.dram[f'w_{name}'][:].rearrange(
            "(k p) n -> p k n", p=P))
        self.wconv[name] = w

    def load_gb(self, name):
        nc = self.nc
        cout = CONV[name][1]
        mt = cout // P
        g = self.p['small'].tile([P, mt, 2], F32, tag=f'gb_{name}', name=f'gb_{name}')
        nc.sync.dma_start(g[:], self.dram[f'gb_{name}'][:].rearrange(
            "(t p) s -> p t s", p=P))
        self.gb[name] = g

    # ---------- conv + stats ----------
    def conv_layer(self, name, src, spack, off):
        """Emit matmuls for all (mtile, chunk), bn stats, raw copies (fp16),
        local (sum, sumsq) written into spack[:, off:off+2*mt]."""
        nc = self.nc
        cin, cout = CONV[name][:2]
        mt = cout // P
        tag = DST_TAG[name]
        tag_bufs = {'act2': 3, 'act4': 2}.get(tag, 1)
        dst = self.p['acts'].tile([P, mt, BL], F16, tag=tag, bufs=tag_bufs,
                                  name=f'a_{name}')
        self.acts[name] = dst

        if name == 'w1':
            wa, wb = self.wconv[name]
            src_lo, src_hi = src
            klist = [(lambda m, _wa=wa: _wa[:, 0, m * P:(m + 1) * P],
                      lambda ch, _s=src_lo: _s[:, 0, ch * CH:(ch + 1) * CH]),
                     (lambda m, _wb=wb: _wb[:, m * P:(m + 1) * P],
                      lambda ch, _s=src_hi: _s[:, ch * CH:(ch + 1) * CH])]
        else:
            w = self.wconv[name]
            kt = cin // P
            klist = [(lambda m, _w=w, _k=k: _w[:, _k, m * P:(m + 1) * P],
                      lambda ch, _s=src, _k=k: _s[:, _k, ch * CH:(ch + 1) * CH])
                     for k in range(kt)]

        st6 = self.p['small'].tile([P, mt, NCH, 6], F32, tag='stats6',
                                   bufs=3, name=f'st6_{name}')
        nk = len(klist)
        H2 = 2 * CH
        for m in range(mt):
            for h in range(NCH // 2):
                ps2 = self.p['psacc'].tile([P, 2, CH], F32, tag='acc', name='ps')
                for k, (wfn, sfn) in enumerate(klist):
                    wap = wfn(m)
                    for c2 in range(2):
                        nc.tensor.matmul(ps2[:, c2, :], wap, sfn(2 * h + c2),
                                         start=(k == 0), stop=(k == nk - 1))
                for c2 in range(2):
                    nc.vector.bn_stats(st6[:, m, 2 * h + c2, :], ps2[:, c2, :])
                eng = self.copy_engine()
                d = dst[:, m, h * H2:(h + 1) * H2]
                if eng is self.nc.vector:
                    eng.tensor_copy(d, ps2[:])
                else:
                    eng.copy(d, ps2[:])

        mv = self.p['small'].tile([P, mt, 2], F32, tag='mv', bufs=2,
                                  name=f'mv_{name}')
        for m in range(mt):
            nc.vector.bn_aggr(mv[:, m, :], st6[:, m, :, :])
        # local sums: sum = mean*BL ; sumsq = (var + mean^2)*BL
        sl = spack[:, off:off + 2 * mt].rearrange("p (m s) -> p m s", s=2)
        t1 = self.p['small'].tile([P, mt], F32, tag='acw', bufs=8, name='t1')
        nc.vector.tensor_mul(t1[:], mv[:, :, 0], mv[:, :, 0])        # mean^2
        nc.vector.tensor_add(t1[:], t1[:], mv[:, :, 1])              # +var
        nc.vector.tensor_scalar_mul(sl[:, :, 1], t1[:], float(BL))   # sumsq
        nc.vector.tensor_scalar_mul(sl[:, :, 0], mv[:, :, 0], float(BL))
        self.spack_off[name] = off
        return off + 2 * mt

    # ---------- AllGather sync ----------
    def ag_sync(self, gi, names, spack, X):
        nc = self.nc
        ccin = self.p['dram'].tile([P, X], F32, tag=f'ccin{gi}', name=f'ccin{gi}')
        ccout = self.p['dram'].tile([N_CORES, P, X], F32, tag=f'ccout{gi}',
                                    addr_space="Shared", name=f'ccout{gi}')
        nc.sync.dma_start(ccin[:], spack[:])
        nc.gpsimd.collective_compute(
            "AllGather", ALU.bypass,
            replica_groups=[list(range(N_CORES))],
            ins=[ccin[:].opt()], outs=[ccout[:].opt()])
        gath = self.p['small'].tile([P, N_CORES, X], F32, tag='gath', bufs=2,
                                    name=f'gath{gi}')
        nc.sync.dma_start(gath[:], ccout[:].rearrange("r p x -> p r x"))
        tot = self.p['small'].tile([P, X], F32, tag='tot', bufs=2,
                                   name=f'tot{gi}')
        nc.vector.tensor_reduce(tot[:], gath[:].rearrange("p r x -> p x r"),
                                axis=mybir.AxisListType.X, op=ALU.add)
        # per-layer a, c
        Bg = float(B)
        for name in names:
            mt = CONV[name][1] // P
            off = self.spack_off[name]
            t3 = tot[:, off:off + 2 * mt].rearrange("p (m s) -> p m s", s=2)
            sm = self.p['small'].tile([P, mt], F32, tag='acw', bufs=8, name='sm')
            nc.vector.tensor_scalar_mul(sm[:], t3[:, :, 0], 1.0 / Bg)   # mean
            sq = self.p['small'].tile([P, mt], F32, tag='acw', bufs=8, name='sq')
            nc.vector.tensor_scalar_mul(sq[:], t3[:, :, 1], 1.0 / Bg)   # E[x^2]
            m2 = self.p['small'].tile([P, mt], F32, tag='acw', bufs=8, name='m2')
            nc.vector.tensor_mul(m2[:], sm[:], sm[:])
            nc.vector.tensor_sub(sq[:], sq[:], m2[:])                   # var
            nc.vector.tensor_scalar_add(sq[:], sq[:], 1e-5)
            nc.scalar.sqrt(sq[:], sq[:])                                # std
            rs = self.p['small'].tile([P, mt], F32, tag='acw', bufs=8, name='rs')
            scr = self.p['small'].tile([P, mt], F32, tag='acw', bufs=8, name='scr')
            nc.vector.reciprocal_approx_accurate(rs[:], sq[:], scr[:])
            ac = self.p['small'].tile([P, mt, 2], F32, tag=f'ac_{name}',
                                      name=f'ac_{name}')
            gb = self.gb[name]
            nc.vector.tensor_mul(ac[:, :, 0], gb[:, :, 0], rs[:])      # a
            nc.vector.tensor_mul(rs[:], ac[:, :, 0], sm[:])            # a*mean
            nc.vector.tensor_sub(ac[:, :, 1], gb[:, :, 1], rs[:])      # c
            self.ac[name] = ac

    def normalize(self, name):
        """in-place relu(a*x + c) on the raw fp16 activations.
        Half-major order so the next layer's first chunk unblocks early."""
        nc = self.nc
        dst = self.acts[name]
        mt = CONV[name][1] // P
        ac = self.ac[name]
        H2 = 2 * CH
        for h in range(NCH // 2):
            for m in range(mt):
                sl = dst[:, m, h * H2:(h + 1) * H2]
                nc.scalar.activation(sl, sl, AF.Relu,
                                     bias=ac[:, m, 1:2], scale=ac[:, m, 0:1])

    def residual_into(self, name, res_src, out_tile=None):
        """out (or res_src buffer) += relu(a*raw + c); raw is acts[name]."""
        nc = self.nc
        raw = self.acts[name]
        mt = CONV[name][1] // P
        ac = self.ac[name]
        target = out_tile if out_tile is not None else res_src
        H2 = 2 * CH
        for h in range(NCH // 2):
            for m in range(mt):
                tmp = self.p['hp'].tile([P, H2], F16, tag='tmp', bufs=2,
                                        name='tmp')
                nc.scalar.activation(tmp[:], raw[:, m, h * H2:(h + 1) * H2],
                                     AF.Relu, bias=ac[:, m, 1:2],
                                     scale=ac[:, m, 0:1])
                sl = target[:, m, h * H2:(h + 1) * H2]
                if out_tile is not None:
                    nc.vector.tensor_add(sl, res_src[:, m, h * H2:(h + 1) * H2],
                                         tmp[:])
                else:
                    nc.vector.tensor_add(sl, sl, tmp[:])


def build(BL_=BL):
    global BL, NCH
    BL, NCH = BL_, BL_ // CH
    nc = bacc.Bacc(None, target_bir_lowering=False, num_devices=N_CORES)

    bld = None
    # ---- DRAM params ----
    tmp_nc = nc
    din = lambda n, s, d=F16: tmp_nc.dram_tensor(n, list(s), d, kind="ExternalInput")
    dout = lambda n, s, d=F32: tmp_nc.dram_tensor(n, list(s), d, kind="ExternalOutput")

    d = {}
    d['e'] = din('e', [P, BL])
    d['wi_lo'] = din('wi_lo', [P, BL])
    d['wi_hi'] = din('wi_hi', [2, BL])
    for name, (cin, cout) in CONV.items():
        if name == 'w1':
            d['w_w1a'] = din('w_w1a', [128, 256])
            d['w_w1b'] = din('w_w1b', [2, 256])
        else:
            d[f'w_{name}'] = din(f'w_{name}', [cin, cout])
        d[f'gb_{name}'] = din(f'gb_{name}', [cout, 2], F32)
    for hname, (dims, _) in HEADS.items():
        for i in range(len(dims) - 1):
            d[f'w_{hname}{i}'] = din(f'w_{hname}{i}', [dims[i], dims[i + 1]])
            d[f'b_{hname}{i}'] = din(f'b_{hname}{i}',
                                     [min(dims[i + 1], P), _cdiv(dims[i + 1], P)], F32)
    d['w_wsm'] = din('w_wsm', [512, 1])
    d['b_wsm'] = din('b_wsm', [1, 1], F32)

    d['o_feat'] = dout('o_feat', [P, 4, BL], F16)
    d['o_link'] = dout('o_link', [2, BL])
    d['o_net'] = dout('o_net', [5, BL])
    d['o_sample'] = dout('o_sample', [2, BL])
    d['o_infl'] = dout('o_infl', [4, BL])

    with tile.TileContext(nc) as tc:
        import contextlib
        with contextlib.ExitStack() as ctx:
            pools = {
                'acts': ctx.enter_context(tc.tile_pool(name='acts', bufs=1)),
                'wp': ctx.enter_context(tc.tile_pool(name='wp', bufs=2)),
                'wh': ctx.enter_context(tc.tile_pool(name='wh', bufs=1)),
                'small': ctx.enter_context(tc.tile_pool(name='small', bufs=1)),
                'hp': ctx.enter_context(tc.tile_pool(name='hp', bufs=2)),
                'psacc': ctx.enter_context(
                    tc.tile_pool(name='psacc', bufs=3, space='PSUM')),
                'psb': ctx.enter_context(
                    tc.tile_pool(name='psb', bufs=1, space='PSUM')),
                'psd': ctx.enter_context(
                    tc.tile_pool(name='psd', bufs=1, space='PSUM')),
                'dram': ctx.enter_context(tc.tile_pool(name='dram', bufs=1,
                                                       space='DRAM')),
                'wsm_p': ctx.enter_context(tc.tile_pool(name='wsm_p', bufs=1)),
            }
            b = Builder(nc, tc, pools)
            b.dram = d
            _build_body(b)
    nc.compile()
    return nc


def _build_body(b):
    nc = b.nc
    pools = b.p
    small, acts, hp = pools['small'], pools['acts'], pools['hp']

    # ---- warmup collective: absorbs ncfw first-call latency during DMAs ----
    wuin = pools['dram'].tile([1, 8], F32, tag='wu_in', name='wu_in')
    wuout = pools['dram'].tile([N_CORES, 1, 8], F32, tag='wu_out',
                               addr_space="Shared", name='wu_out')
    wtmp = small.tile([1, 8], F32, tag='wu_sb', name='wtmp')
    nc.gpsimd.memset(wtmp[:], 0.0)
    nc.sync.dma_start(wuin[:], wtmp[:])
    nc.gpsimd.collective_compute(
        "AllGather", ALU.bypass, replica_groups=[list(range(N_CORES))],
        ins=[wuin[:].opt()], outs=[wuout[:].opt()])

    # ---- constants ----
    ones_c = small.tile([8, 1], F32, tag='ones_c', name='ones_c')
    nc.gpsimd.memset(ones_c[:], 1.0)
    ones_r16 = small.tile([1, P], F16, tag='ones_r16', name='ones_r16')
    nc.gpsimd.memset(ones_r16[:], 1.0)
    ones_r32 = small.tile([1, P], F32, tag='ones_r32', name='ones_r32')
    nc.gpsimd.memset(ones_r32[:], 1.0)

    # ---- head weights + biases (resident) ----
    whead = {}
    bhead = {}
    for hname, (dims, _) in HEADS.items():
        for i in range(len(dims) - 1):
            cin, cout = dims[i], dims[i + 1]
            kt = _cdiv(cin, P)
            if cin >= P:
                w = pools['wh'].tile([P, kt, cout], F16, tag=f'wh_{hname}{i}',
                                     name=f'wh_{hname}{i}')
                nc.sync.dma_start(w[:], b.dram[f'w_{hname}{i}'][:].rearrange(
                    "(k p) n -> p k n", p=P))
            else:
                w = pools['wh'].tile([cin, cout], F16, tag=f'wh_{hname}{i}',
                                     name=f'wh_{hname}{i}')
                nc.sync.dma_start(w[:], b.dram[f'w_{hname}{i}'][:])
            whead[(hname, i)] = w
            bb = pools['wh'].tile([min(cout, P), _cdiv(cout, P)], F32,
                                  tag=f'bh_{hname}{i}', name=f'bh_{hname}{i}')
            nc.sync.dma_start(bb[:], b.dram[f'b_{hname}{i}'][:])
            bhead[(hname, i)] = bb
    w_wsm = pools['wh'].tile([P, 4, 1], F16, tag='w_wsm', name='w_wsm')
    nc.sync.dma_start(w_wsm[:], b.dram['w_wsm'][:].rearrange(
        "(k p) n -> p k n", p=P))
    b_wsm = pools['wh'].tile([1, 1], F32, tag='b_wsm', name='b_wsm')
    nc.sync.dma_start(b_wsm[:], b.dram['b_wsm'][:])

    # ---- gamma/beta tables ----
    for name in CONV:
        b.load_gb(name)

    # ---- inputs ----
    e = acts.tile([P, 1, BL], F16, tag='ex', name='e')
    nc.sync.dma_start(e[:, 0, :], b.dram['e'][:])
    wi_lo = acts.tile([P, 1, BL], F16, tag='wi', name='wi_lo')
    nc.sync.dma_start(wi_lo[:, 0, :], b.dram['wi_lo'][:])
    wi_hi = acts.tile([2, BL], F16, tag='wihi', name='wi_hi')
    nc.sync.dma_start(wi_hi[:], b.dram['wi_hi'][:])

    # ================= group 1: wave + w1 =================
    spgrp = {}

    def new_spack(gi, X):
        sp = small.tile([P, X], F32, tag='spack', bufs=2, name=f'spack{gi}')
        spgrp[gi] = sp
        return sp

    def group_X(names):
        return sum(2 * (CONV[n][1] // P) for n in names)

    gi = 0
    names = AG_GROUPS[gi]
    X = group_X(names)
    sp = new_spack(gi, X)
    b.load_conv_weight('wave')
    off = b.conv_layer('wave', e, sp, 0)
    b.load_conv_weight('w1')
    off = b.conv_layer('w1', (wi_lo, wi_hi), sp, off)
    b.ag_sync(gi, names, sp, X)
    b.normalize('wave')
    # x = e + 0.3*wf  (in place into e)
    wf = b.acts['wave']
    nc.vector.scalar_tensor_tensor(e[:, 0, :], wf[:, 0, :], 0.3, e[:, 0, :],
                                   op0=ALU.mult, op1=ALU.add)
    x = e
    b.normalize('w1')

    # ================= group 2: g1 t1 w2 =================
    gi = 1
    names = AG_GROUPS[gi]
    X = group_X(names)
    sp = new_spack(gi, X)
    b.load_conv_weight('g1')
    off = b.conv_layer('g1', x, sp, 0)
    b.load_conv_weight('t1')
    off = b.conv_layer('t1', x, sp, off)
    b.load_conv_weight('w2')
    off = b.conv_layer('w2', b.acts['w1'], sp, off)
    b.ag_sync(gi, names, sp, X)
    b.normalize('g1')
    b.normalize('t1')
    b.normalize('w2')

    # ---- wsm head: wo0 = sigmoid(z0 - z1), weights host-folded to one logit ----
    wo0_sb = acts.tile([1, BL], F32, tag='wo', name='wo0_sb')
    w2n = b.acts['w2']
    for ch in range(NCH):
        csl = slice(ch * CH, (ch + 1) * CH)
        ps = pools['psacc'].tile([1, CH], F32, tag='acc', name='ps_wsm')
        for k in range(4):
            nc.tensor.matmul(ps[:], w_wsm[:, k, :], w2n[:, k, csl],
                             start=(k == 0), stop=(k == 3))
        nc.scalar.activation(wo0_sb[:, csl], ps[:], AF.Sigmoid,
                             bias=b_wsm[:, 0:1], scale=1.0)

    # ================= group 3: g2 t2 =================
    gi = 2
    names = AG_GROUPS[gi]
    X = group_X(names)
    sp = new_spack(gi, X)
    b.load_conv_weight('g2')
    off = b.conv_layer('g2', b.acts['g1'], sp, 0)
    b.load_conv_weight('t2')
    off = b.conv_layer('t2', b.acts['t1'], sp, off)
    b.ag_sync(gi, names, sp, X)
    b.normalize('g2')
    b.normalize('t2')          # -> tf

    # ================= group 4: g3 ; r2 =================
    gi = 3
    names = AG_GROUPS[gi]
    X = group_X(names)
    sp = new_spack(gi, X)
    b.load_conv_weight('g3')
    off = b.conv_layer('g3', b.acts['g2'], sp, 0)
    b.load_conv_weight('r2')
    off = b.conv_layer('r2', b.acts['t2'], sp, off)
    b.ag_sync(gi, names, sp, X)
    b.normalize('g3')
    # t_final = tf + relu(a*r2 + c)   (in place into tf)
    b.residual_into('r2', b.acts['t2'])
    tfin = b.acts['t2']

    # ================= groups 5..9: g4..g8 =================
    prev = 'g3'
    for gi, name in zip(range(4, 9), ['g4', 'g5', 'g6', 'g7', 'g8']):
        X = group_X((name,))
        sp = new_spack(gi, X)
        b.load_conv_weight(name)
        b.conv_layer(name, b.acts[prev], sp, 0)
        b.ag_sync(gi, (name,), sp, X)
        b.normalize(name)
        prev = name
    gf = b.acts['g8']

    # ================= group 10: r1 =================
    gi = 9
    X = group_X(('r1',))
    sp = new_spack(gi, X)
    b.load_conv_weight('r1')
    b.conv_layer('r1', gf, sp, 0)
    b.ag_sync(gi, ('r1',), sp, X)
    # feat = gf + relu(a*r1 + c)  (into fresh feat tile)
    feat = acts.tile([P, 4, BL], F16, tag='feat', name='feat')
    b.residual_into('r1', gf, out_tile=feat)

    # ================= fusion + heads per chunk =================
    for ch in range(NCH):
        csl = slice(ch * CH, (ch + 1) * CH)
        # broadcast wo0 over partitions via ones-matmul
        bc0 = pools['psb'].tile([P, CH], F32, tag='bc', name='bc0')
        nc.tensor.matmul(bc0[:], ones_r32[0:1, :], wo0_sb[0:1, csl],
                         start=True, stop=True)
        # feature = tf + wo0*(gfinal - tf)   (wo1 == 1-wo0 for 2-way softmax)
        for m in range(4):
            fsl = feat[:, m, csl]
            tsl = tfin[:, m, csl]
            nc.vector.tensor_sub(fsl, fsl, tsl)
            nc.vector.scalar_tensor_tensor(fsl, fsl, 1.0, bc0[:],
                                           op0=ALU.mult, op1=ALU.mult)
            nc.vector.tensor_add(fsl, fsl, tsl)

        # ---- heads on this chunk ----
        for hname, (dims, final) in HEADS.items():
            cur = feat
            cur_sl = csl
            nlay = len(dims) - 1
            for i in range(nlay):
                cin, cout = dims[i], dims[i + 1]
                kt = _cdiv(cin, P)
                mt = _cdiv(cout, P)
                w = whead[(hname, i)]
                bb = bhead[(hname, i)]
                last = (i == nlay - 1)
                if not last:
                    h = hp.tile([min(cout, P), mt, CH], F16,
                                tag=f'h{min(cout, P)}_{mt}', bufs=2,
                                name=f'h_{hname}{i}')
                for m in range(mt):
                    pp = min(P, cout - m * P)
                    ps = pools['psacc'].tile([pp, CH], F32, tag='acc',
                                             name=f'ps_{hname}{i}')
                    for k in range(kt):
                        if cin >= P:
                            wsl = w[:, k, m * P:m * P + pp]
                            xsl = cur[:, k, cur_sl]
                        else:
                            wsl = w[:, m * P:m * P + pp]
                            xsl = cur[:, 0, cur_sl] if cur.shape[1] > 1 else \
                                cur[:, cur_sl] if len(cur.shape) == 2 else \
                                cur[:, 0, cur_sl]
                        nc.tensor.matmul(ps[:], wsl, xsl,
                                         start=(k == 0), stop=(k == kt - 1))
                    if not last:
                        nc.scalar.activation(h[:pp, m, :], ps[:], AF.Relu,
                                             bias=bb[:pp, m:m + 1], scale=1.0)
                    else:
                        if final == 'sigmoid2':
                            sm = hp.tile([2, CH], F32, tag='sm', bufs=1,
                                         name=f'sm_{hname}')
                            nc.scalar.activation(sm[:], ps[:], AF.Sigmoid,
                                                 bias=bb[:2, 0:1], scale=1.0)
                            nc.sync.dma_start(b.dram[f'o_{hname}'][:, csl], sm[:])
                        elif final == 'softmax':
                            M = cout
                            ez = hp.tile([M, CH], F32, tag='ez', bufs=1,
                                         name=f'ez_{hname}')
                            nc.scalar.activation(ez[:], ps[:], AF.Exp,
                                                 bias=bb[:M, 0:1], scale=1.0)
                            den = pools['psd'].tile([1, CH], F32, tag='den',
                                                    name='den_h')
                            nc.tensor.matmul(den[:], ones_c[0:M, 0:1], ez[:],
                                             start=True, stop=True)
                            rec = hp.tile([1, CH], F32, tag='rec', bufs=1,
                                          name='rec_h')
                            nc.vector.reciprocal(rec[:], den[:])
                            bcp = pools['psb'].tile([M, CH], F32, tag='bc',
                                                    name='bc_h')
                            nc.tensor.matmul(bcp[:], ones_r32[0:1, 0:M], rec[:],
                                             start=True, stop=True)
                            sm = hp.tile([M, CH], F32, tag='sm', bufs=1,
                                         name=f'sm_{hname}')
                            nc.vector.tensor_mul(sm[:], ez[:], bcp[:])
                            nc.sync.dma_start(b.dram[f'o_{hname}'][:, csl], sm[:])
                        else:
                            oo = hp.tile([cout, CH], F32, tag='io', bufs=1,
                                         name=f'io_{hname}')
                            nc.scalar.activation(oo[:], ps[:], AF.Identity,
                                                 bias=bb[:cout, 0:1], scale=1.0)
                            nc.sync.dma_start(b.dram[f'o_{hname}'][:, csl], oo[:])
                if not last:
                    cur = h
                    cur_sl = slice(0, CH)

    # feature out
    for m in range(4):
        nc.sync.dma_start(b.dram['o_feat'][:, m, :], feat[:, m, :])


# ======================= host side =======================

def _prep_inputs(inputs):
    """Fold/transposes/shard on the host; returns in_maps list."""
    f16 = np.float16

    def npf(x):
        return np.asarray(x, dtype=np.float32)

    e = npf(inputs['node_embedding'])[:, 0, :]      # [B, 128]
    wi = npf(inputs['weight_input'])[:, 0, :]       # [B, 130]
    IN = e.shape[1]

    H = np.zeros((IN // 2, IN), np.float32)
    for j in range(IN // 2):
        H[j, 2 * j] = H[j, 2 * j + 1] = 1.0 / np.sqrt(2.0, dtype=np.float32)

    conv_params = {
        'wave': inputs['wave_params'][0],
        'w1': inputs['weight_params'][0], 'w2': inputs['weight_params'][1],
        'g1': inputs['gen_params'][0], 'g2': inputs['gen_params'][1],
        'g3': inputs['gen_params'][2], 'g4': inputs['gen_params'][3],
        'g5': inputs['gen_params'][4], 'g6': inputs['gen_params'][5],
        'g7': inputs['gen_params'][6], 'g8': inputs['gen_params'][7],
        't1': inputs['tgt_params'][0], 't2': inputs['tgt_params'][1],
        'r1': inputs['res1_params'][0], 'r2': inputs['res2_params'][0],
    }

    shared = {}
    for name, (W, bias, g, beta) in conv_params.items():
        W = npf(W)
        k = W.shape[-1]
        Weff = W[:, :, (k - 1) // 2]                 # [cout, cin]
        if name == 'wave':
            Weff = Weff @ H                          # fold Haar
        lhsT = np.ascontiguousarray(Weff.T).astype(f16)   # [cin, cout]
        if name == 'w1':
            shared['w_w1a'] = lhsT[:128]
            shared['w_w1b'] = lhsT[128:130]
        else:
            shared[f'w_{name}'] = lhsT
        shared[f'gb_{name}'] = np.ascontiguousarray(
            np.stack([npf(g), npf(beta)], axis=1))   # [cout, 2]

    head_params = {
        'link': inputs['link_mlp'], 'net': inputs['net_mlp'],
        'sample': inputs['sample_mlp'], 'infl': inputs['infl_mlp'],
    }
    for hname, params in head_params.items():
        for i, (W, bias) in enumerate(params):
            W = npf(W)
            shared[f'w_{hname}{i}'] = np.ascontiguousarray(W.T).astype(f16)
            cout = W.shape[0]
            pp = min(cout, P)
            nt = _cdiv(cout, P)
            bp = np.zeros((pp, nt), np.float32)
            bias = npf(bias)
            for t in range(nt):
                seg = bias[t * P:(t + 1) * P]
                bp[:len(seg), t] = seg
            shared[f'b_{hname}{i}'] = bp
    Wsm, bsm = inputs['wsm_params'][0]
    Wsm, bsm = npf(Wsm), npf(bsm)
    shared['w_wsm'] = np.ascontiguousarray(
        (Wsm[0] - Wsm[1]).reshape(512, 1)).astype(f16)
    shared['b_wsm'] = np.array([[bsm[0] - bsm[1]]], np.float32)
    # 2-way softmax == sigmoid of folded logit difference (exact)
    for hname in ('link', 'sample'):
        i = len(head_params[hname]) - 1
        W, bias = head_params[hname][i]
        W, bias = npf(W), npf(bias)
        Wd = np.stack([W[0] - W[1], W[1] - W[0]], axis=1)   # [cin, 2]
        bd = np.array([[bias[0] - bias[1]], [bias[1] - bias[0]]], np.float32)
        shared[f'w_{hname}{i}'] = np.ascontiguousarray(Wd).astype(f16)
        shared[f'b_{hname}{i}'] = bd.reshape(2, 1)

    in_maps = []
    for c in range(N_CORES):
        sl = slice(c * BL, (c + 1) * BL)
        m = dict(shared)
        m['e'] = np.ascontiguousarray(e[sl].T).astype(f16)
        wi_c = wi[sl]
        m['wi_lo'] = np.ascontiguousarray(wi_c[:, :128].T).astype(f16)
        m['wi_hi'] = np.ascontiguousarray(wi_c[:, 128:130].T).astype(f16)
        in_maps.append(m)
    return in_maps


_CACHED_NC = None
TRACE = False
LAST_RESULTS = None


def kernel(**inputs):
    global _CACHED_NC, LAST_RESULTS
    in_maps = _prep_inputs(inputs)
    if _CACHED_NC is None:
        _CACHED_NC = build()
    nc = _CACHED_NC
    res = run_bass_kernel_spmd(nc, in_maps, core_ids=list(range(N_CORES)),
                               trace=TRACE)
    LAST_RESULTS = res

    link = np.zeros((B, 2), np.float32)
    net = np.zeros((B, 5), np.float32)
    sample = np.zeros((B, 2), np.float32)
    feature = np.zeros((B, 512), np.float32)
    infl = np.zeros((B, 4), np.float32)
    for c, out in enumerate(res.results):
        sl = slice(c * BL, (c + 1) * BL)
        link[sl] = out['o_link'].T
        net[sl] = out['o_net'].T
        sample[sl] = out['o_sample'].T
        infl[sl] = out['o_infl'].T
        f = out['o_feat'].astype(np.float32)          # [128, 4, BL]
        feature[sl] = f.transpose(2, 1, 0).reshape(BL, 512)
    return (link, net, sample, feature, infl)


if __name__ == '__main__':
    nc = build()
    print("built ok")
